# revision 26
# baseline (speedup 1.0000x reference)
"""CompressKV gating kernel for 8 Trainium2 NeuronCores (v3).

Reference computation (per batch b, head h):
    x_s = x[b, :, h, :]                                  # [N=4096, D=128]
    windows n = 0..254, rows r = 16n + k, k = 0..31
    logits[n, g] = sum_{k,d} x_s[16n+k, d] * W[g, k, d]  # W = W_gate.reshape(32,32,128)
    gate = softmax_g(logits)
    out[n, d] = sum_k gate[n, k] * x_s[16n+k, d]

Sharding: B*H = 32 (b,h) slices, 4 per core, no cross-core communication.
Host pre-packs x per core in two bf16 layouts:
  xn : row-chunked native   [4, 128(p), 32(c)*128(d)]  (chunk c = rows 128c+p)
  xtm: d-major m-interleaved [4, 128(d), 16(m)*256(c)] (col 256m+c = row 16c+m)
plus wt[d, 64m+32kap+g] = W_gate[g, (m+16kap)*128+d].

v3 vs the 56us baseline:
  - stage A packs (kappa,gate)=64 outputs per streamed column via two
    interleaved PE column-tile accumulation chains; the m-interleaved
    xtm layout makes every rhs stream contiguous: 16 matmuls x 256 cols
    per slice (half the baseline's streamed columns).
    logits[n,g] = acc[(0,g),n] + acc[(1,g),n+1] folded as exp-products.
  - exp relocates the 4 (chain,kappa) psum blocks to partition base 0
    (DVE tensor_tensor requires equal SBUF base partitions).
  - bf16 outputs leave in one contiguous DMA per slice as it finishes.
  - input dispatch split across both HWDGE rings (SP + ACT).
  - PE warm-up matmuls during the initial DMA fill raise the p-state.
"""

import sys

import numpy as np

for _p in ("/opt/trn_rl_repo", "/opt/pypackages"):
    if _p not in sys.path:
        sys.path.append(_p)

import ml_dtypes

_B, _N, _H, _D = 2, 4096, 16, 128
_K = 32          # window (kernel) size
_ST = 16         # stride
_NB = 255        # num windows
_NC = 8          # cores
_SL = 4          # (b,h) slices per core
_NCH = 32        # 128-row chunks per slice
_NWARM = 16      # PE p-state warm-up matmuls

_prog_cache = {}


def _build_program():
    import concourse.mybir as mybir
    from concourse import bacc, masks, tile

    f32 = mybir.dt.float32
    bf16 = mybir.dt.bfloat16

    nc = bacc.Bacc()
    xn = nc.dram_tensor("xn", [_SL, 128, _N], bf16, kind="ExternalInput")
    xtm = nc.dram_tensor("xtm", [_SL, 128, _N], bf16, kind="ExternalInput")
    wt = nc.dram_tensor("wt", [128, 1024], bf16, kind="ExternalInput")
    # out[s, p, half, d] = out_slice_s[window 128*half + p, d]
    out = nc.dram_tensor("out", [_SL, 128, 2, _D], bf16, kind="ExternalOutput")

    with tile.TileContext(nc) as tc:
        with (
            tc.tile_pool(name="const", bufs=1) as cpool,
            tc.tile_pool(name="data", bufs=2) as dpool,
            tc.tile_pool(name="small", bufs=2) as spool,
            tc.tile_pool(name="psA", bufs=2, space="PSUM") as psa_pool,
            tc.tile_pool(name="psC", bufs=2, space="PSUM") as psc_pool,
            tc.tile_pool(name="psM", bufs=1, space="PSUM") as psm_pool,
            tc.tile_pool(name="psS", bufs=1, space="PSUM") as pss_pool,
            tc.tile_pool(name="psND", bufs=1, space="PSUM") as psnd_pool,
            tc.tile_pool(name="psW", bufs=1, space="PSUM") as psw_pool,
        ):
            wmup = cpool.tile([128, 128], bf16)
            nc.vector.memset(wmup[:], 0.5)
            ones32 = cpool.tile([32, 1], bf16)
            nc.vector.memset(ones32[:], 1.0)
            ident = cpool.tile([128, 128], f32)
            masks.make_identity(nc, ident[:])
            # band master: mband[k, c] = 1 iff c == k + 128.  Slicing cols
            # [144-16j : 272-16j] gives the [32k, 128r] band matrix with
            # 1 at r == k + 16j - 16 (rows outside [0,128) auto-dropped).
            mband = cpool.tile([32, 272], bf16)
            nc.gpsimd.memset(mband[:], 0.0)
            nc.gpsimd.affine_select(
                out=mband[:],
                in_=mband[:],
                compare_op=mybir.AluOpType.not_equal,
                fill=1.0,
                base=128,
                pattern=[[-1, 272]],
                channel_multiplier=1,
            )
            wt_sb = cpool.tile([128, 1024], bf16)
            nc.scalar.dma_start(wt_sb[:], wt[:, :])

            # PE p-state warm-up: harmless matmuls while input DMAs fill
            psW = psw_pool.tile([64, 128], f32)
            for _ in range(_NWARM):
                nc.tensor.matmul(
                    psW[:, :], wmup[:, 0:64], wmup[:, :],
                    start=True, stop=True, skip_group_check=True,
                )

            # queue input DMAs on the SP ring (pure DMA ring: backpressure
            # waits must not block compute dispatch).  With bufs=2 input
            # pools, slice s+2's DMA naturally waits for slice s's last
            # reader — keeping HBM arrivals slice-ordered and just-in-time
            # instead of all transfers round-robin-finishing together.
            xtm_sb = [
                dpool.tile([128, _N], bf16, tag="xtm", name=f"xtm_{s}")
                for s in range(_SL)
            ]
            xn_sb = [
                dpool.tile([128, _N], bf16, tag="xn", name=f"xn_{s}")
                for s in range(_SL)
            ]
            # half-granular transfers: the ~0.6-0.9us per-dispatch sequencer
            # cost paces transfers to ~2 in flight at full bandwidth, and
            # consumers start on the first half via region-based deps
            def dma_half(dst, src, s, h):
                nc.sync.dma_start(
                    dst[s][:, 2048 * h : 2048 * h + 2048],
                    src[s, :, 2048 * h : 2048 * h + 2048],
                )

            dma_half(xtm_sb, xtm, 0, 0)
            dma_half(xtm_sb, xtm, 0, 1)
            dma_half(xn_sb, xn, 0, 0)
            dma_half(xtm_sb, xtm, 1, 0)
            dma_half(xn_sb, xn, 0, 1)
            dma_half(xtm_sb, xtm, 1, 1)
            dma_half(xn_sb, xn, 1, 0)
            dma_half(xtm_sb, xtm, 2, 0)
            dma_half(xn_sb, xn, 1, 1)
            dma_half(xtm_sb, xtm, 2, 1)
            dma_half(xn_sb, xn, 2, 0)
            dma_half(xtm_sb, xtm, 3, 0)
            dma_half(xn_sb, xn, 2, 1)
            dma_half(xtm_sb, xtm, 3, 1)
            dma_half(xn_sb, xn, 3, 0)
            dma_half(xn_sb, xn, 3, 1)

            fin = {}

            def emit_finish_copy(s):
                """psC -> SBUF f32 copy (DVE), frees psC."""
                psC, outT_sb, rden = fin[s]
                nc.vector.tensor_copy(outT_sb[:, 0:255], psC[:, 0:255])

            def emit_finish_rest(s):
                """transpose to [n, d] -> normalize -> slice DMA out."""
                _, outT_sb, rden = fin.pop(s)
                psND = psnd_pool.tile([128, 256], f32, tag="psND")
                nc.tensor.transpose(
                    psND[0:128, 0:128], outT_sb[:, 0:128], ident[:, :]
                )
                nc.tensor.transpose(
                    psND[0:127, 128:256], outT_sb[:, 128:255], ident[:, :]
                )
                o_s = spool.tile([128, 2, _D], bf16, tag="o_s")
                nc.gpsimd.memset(o_s[:], 0.0)
                nc.vector.tensor_scalar(
                    o_s[:, 0, :], psND[0:128, 0:128], rden[0:128, 0:1],
                    None, mybir.AluOpType.mult,
                )
                nc.vector.tensor_scalar(
                    o_s[0:127, 1, :], psND[0:127, 128:256],
                    rden[0:127, 1:2], None, mybir.AluOpType.mult,
                )
                nc.scalar.dma_start(out[s, :, :, :], o_s[:, :, :])

            for s in range(_SL):
                if s > 0:
                    emit_finish_copy(s - 1)

                # ---- stage A: 2 concurrent accumulation chains, 64-wide ----
                # psA[64*ch + 32*kap + g, c] += wt[:,64m+32kap+g] . x[16c+m, :]
                psA = psa_pool.tile([128, 256], f32, tag="psA", name=f"psA_{s}")
                for m in range(16):
                    ch = m & 1
                    nc.tensor.matmul(
                        psA[64 * ch : 64 * ch + 64, :],
                        wt_sb[:, 64 * m : 64 * m + 64],
                        xtm_sb[s][:, 256 * m : 256 * m + 256],
                        start=(m < 2),
                        stop=(m >= 14),
                        tile_position=(0, 64 * ch),
                        skip_group_check=True,
                    )

                # previous slice's PE transposes fill the exp/fold latency gap
                if s > 0:
                    emit_finish_rest(s - 1)

                # ---- stage B: exp + fold + denominators ----
                # relocate the 4 (chain,kappa) psum blocks to partition base 0
                # during exp (DVE tensor_tensor needs equal SBUF bases)
                E = spool.tile([32, 1024], f32, tag="E")
                for q in range(4):
                    nc.scalar.activation(
                        E[:, 256 * q : 256 * q + 256],
                        psA[32 * q : 32 * q + 32, :],
                        mybir.ActivationFunctionType.Exp,
                    )

                psC = psc_pool.tile([128, 256], f32, tag="psC", name=f"psC_{s}")
                nc.vector.memset(psC[:], 0.0)

                # kappa products across chains: psA blocks are (ch,kap) =
                # q0:(0,0) q1:(0,1) q2:(1,0) q3:(1,1)
                P = spool.tile([32, 512], f32, tag="P")
                nc.vector.tensor_mul(P[:, 0:256], E[:, 0:256], E[:, 512:768])
                nc.vector.tensor_mul(P[:, 256:512], E[:, 256:512], E[:, 768:1024])
                e_kn = spool.tile([32, 256], bf16, tag="e_kn")
                nc.vector.tensor_mul(
                    e_kn[:, 0:255], P[:, 0:255], P[:, 257:512]
                )

                # denominators: ones-matmul row + PE transpose + reciprocal
                psM = psm_pool.tile([128, 258], f32, tag="psM")
                nc.tensor.matmul(
                    psM[0:1, 0:255], ones32[:, 0:1], e_kn[:, 0:255],
                    start=True, stop=True, skip_group_check=True,
                )
                den_sb = spool.tile([1, 256], f32, tag="den")
                nc.vector.tensor_copy(den_sb[0:1, 0:255], psM[0:1, 0:255])
                nc.tensor.transpose(
                    psM[0:128, 256:257], den_sb[0:1, 0:128], ident[0:1, 0:1]
                )
                nc.tensor.transpose(
                    psM[0:127, 257:258], den_sb[0:1, 128:255], ident[0:1, 0:1]
                )
                rden = spool.tile([128, 2], f32, tag="rden")
                nc.vector.reciprocal(rden[0:128, 0:1], psM[0:128, 256:257])
                nc.vector.reciprocal(rden[0:127, 1:2], psM[0:127, 257:258])

                # ---- S band matrix on PE: S[16j-16+k, 32j+c] = e[k, 8c+j-1]
                psS = pss_pool.tile([128, 288], f32, tag="psS")
                for j in range(9):
                    c0 = 1 if j == 0 else 0
                    c1 = 31 if j == 8 else 32
                    nc.tensor.matmul(
                        psS[:, 32 * j + c0 : 32 * j + c1],
                        mband[:, 144 - 16 * j : 272 - 16 * j],
                        e_kn[:, 8 * c0 + j - 1 : 8 * (c1 - 1) + j : 8],
                        start=True,
                        stop=True,
                        skip_group_check=True,
                    )
                S_sb = spool.tile([128, 288], bf16, tag="S")
                nc.vector.tensor_copy(S_sb[:, 1:287], psS[:, 1:287])

                # ---- stage C: psC[d, n] += xn_chunk^T @ S_cols ----
                for c in range(_NCH):
                    j0 = 1 if c == 0 else 0
                    j1 = 8 if c == _NCH - 1 else 9
                    nc.tensor.matmul(
                        psC[:, 8 * c - 1 + j0 : 8 * c - 1 + j1],
                        xn_sb[s][:, 128 * c : 128 * c + 128],
                        S_sb[:, 32 * j0 + c : 32 * (j1 - 1) + c + 1 : 32],
                        start=False,
                        stop=(c == _NCH - 1),
                        skip_group_check=True,
                    )

                outT_sb = spool.tile([128, 256], f32, tag="outT")
                fin[s] = (psC, outT_sb, rden)

            emit_finish_copy(_SL - 1)
            emit_finish_rest(_SL - 1)

    nc.compile()
    return nc


def _get_program():
    if "nc" not in _prog_cache:
        _prog_cache["nc"] = _build_program()
    return _prog_cache["nc"]


def _host_inputs(x, W_gate):
    bf16 = ml_dtypes.bfloat16
    x = np.asarray(x, dtype=np.float32)
    W = np.asarray(W_gate, dtype=np.float32)
    # wt[d, 64m + 32kap + g] = W_gate[g, (m+16kap)*128 + d]
    wt_host = np.ascontiguousarray(
        W.reshape(_K, 2, 16, _D).transpose(3, 2, 1, 0).reshape(_D, 1024)
    ).astype(bf16)
    xb = x.astype(bf16)
    in_maps = []
    for core in range(_NC):
        xn = np.empty((_SL, 128, _N), dtype=bf16)
        xtm = np.empty((_SL, 128, _N), dtype=bf16)
        for si in range(_SL):
            p = core * _SL + si
            b, h = p // _H, p % _H
            xs = xb[b, :, h, :]  # [4096, 128]
            # xn[si, p_, 128c + d] = x[128c + p_, d]
            xn[si] = (
                xs.reshape(_NCH, 128, _D).transpose(1, 0, 2).reshape(128, -1)
            )
            # xtm[si, d, 256m + c] = x[16c + m, d]
            xtm[si] = (
                xs.reshape(256, 16, _D).transpose(2, 1, 0).reshape(128, -1)
            )
        in_maps.append({"xn": xn, "xtm": xtm, "wt": wt_host})
    return in_maps


def _assemble(results):
    out = np.empty((_B, _NB, _H, _D), dtype=np.float32)
    for core in range(_NC):
        o = np.asarray(results[core]["out"]).astype(np.float32)  # [4,128,2,128]
        for si in range(_SL):
            p = core * _SL + si
            b, h = p // _H, p % _H
            out[b, 0:128, h, :] = o[si, :, 0, :]
            out[b, 128:_NB, h, :] = o[si, 0:127, 1, :]
    return out


def _install_trace_hooks():
    """Shim the axon NTFF profile hook (missing in this image) so
    run_bass_kernel_spmd(trace=True) can collect a HW profile, and neuter
    the artifact upload (zero-egress container)."""
    import contextlib
    import ctypes
    import types

    try:
        from antenv.axon_hooks import get_axon_ntff_profile_hook  # noqa: F401

        return
    except ImportError:
        pass

    lib = ctypes.CDLL("/opt/axon/libaxon_pjrt.so")
    if not hasattr(lib, "axon_start_nrt_profile"):
        return
    lib.axon_start_nrt_profile.argtypes = [
        ctypes.POINTER(ctypes.c_int64),
        ctypes.c_size_t,
    ]
    lib.axon_start_nrt_profile.restype = ctypes.c_int64
    lib.axon_stop_nrt_profile.argtypes = [ctypes.c_char_p]
    lib.axon_stop_nrt_profile.restype = ctypes.c_int64

    @contextlib.contextmanager
    def _hook(output_dir, device_ids):
        import jax

        jax.devices()
        if device_ids:
            ids = (ctypes.c_int64 * len(device_ids))(*device_ids)
            rc = lib.axon_start_nrt_profile(ids, len(device_ids))
        else:
            rc = lib.axon_start_nrt_profile(None, 0)
        if rc != 0:
            raise RuntimeError(f"axon_start_nrt_profile rc={rc}")
        try:
            yield
        finally:
            n = lib.axon_stop_nrt_profile(str(output_dir).encode())
            print(f"profile: {n} file(s) written to {output_dir}")

    mod = types.ModuleType("antenv.axon_hooks")
    mod.get_axon_ntff_profile_hook = lambda: _hook
    mod.set_axon_ntff_profile_hook = lambda h: None
    sys.modules["antenv.axon_hooks"] = mod

    from concourse import bass_utils as bu

    bu.upload_artifacts = lambda tmpdir: tmpdir


def run(x, W_gate, trace=False, **kw):
    from concourse.bass_utils import run_bass_kernel_spmd

    if trace:
        _install_trace_hooks()
    nc = _get_program()
    in_maps = _host_inputs(x, W_gate)
    res = run_bass_kernel_spmd(nc, in_maps, list(range(_NC)), trace=trace, **kw)
    return _assemble(res.results), res


def kernel(x, W_gate):
    out, _ = run(x, W_gate)
    return out


# revision 28
# speedup vs baseline: 1.1271x; 1.1271x over previous
"""CompressKV gating kernel for 8 Trainium2 NeuronCores (v3).

Reference computation (per batch b, head h):
    x_s = x[b, :, h, :]                                  # [N=4096, D=128]
    windows n = 0..254, rows r = 16n + k, k = 0..31
    logits[n, g] = sum_{k,d} x_s[16n+k, d] * W[g, k, d]  # W = W_gate.reshape(32,32,128)
    gate = softmax_g(logits)
    out[n, d] = sum_k gate[n, k] * x_s[16n+k, d]

Sharding: B*H = 32 (b,h) slices, 4 per core, no cross-core communication.
Host pre-packs x per core in two bf16 layouts:
  xn : row-chunked native   [4, 128(p), 32(c)*128(d)]  (chunk c = rows 128c+p)
  xtm: d-major m-interleaved [4, 128(d), 16(m)*256(c)] (col 256m+c = row 16c+m)
plus wt[d, 64m+32kap+g] = W_gate[g, (m+16kap)*128+d].

v3 vs the 56us baseline:
  - stage A packs (kappa,gate)=64 outputs per streamed column via two
    interleaved PE column-tile accumulation chains; the m-interleaved
    xtm layout makes every rhs stream contiguous: 16 matmuls x 256 cols
    per slice (half the baseline's streamed columns).
    logits[n,g] = acc[(0,g),n] + acc[(1,g),n+1] folded as exp-products.
  - exp relocates the 4 (chain,kappa) psum blocks to partition base 0
    (DVE tensor_tensor requires equal SBUF base partitions).
  - bf16 outputs leave in one contiguous DMA per slice as it finishes.
  - input dispatch split across both HWDGE rings (SP + ACT).
  - PE warm-up matmuls during the initial DMA fill raise the p-state.
"""

import sys

import numpy as np

for _p in ("/opt/trn_rl_repo", "/opt/pypackages"):
    if _p not in sys.path:
        sys.path.append(_p)

import ml_dtypes

_B, _N, _H, _D = 2, 4096, 16, 128
_K = 32          # window (kernel) size
_ST = 16         # stride
_NB = 255        # num windows
_NC = 8          # cores
_SL = 4          # (b,h) slices per core
_NCH = 32        # 128-row chunks per slice
_NWARM = 32      # PE p-state warm-up matmuls

_prog_cache = {}


def _build_program():
    import concourse.mybir as mybir
    from concourse import bacc, masks, tile

    f32 = mybir.dt.float32
    bf16 = mybir.dt.bfloat16

    nc = bacc.Bacc()
    xn = nc.dram_tensor("xn", [_SL, 128, _N], bf16, kind="ExternalInput")
    xtm = nc.dram_tensor("xtm", [_SL, 128, _N], bf16, kind="ExternalInput")
    wt = nc.dram_tensor("wt", [128, 1024], bf16, kind="ExternalInput")
    # out[s, p, half, d] = out_slice_s[window 128*half + p, d]
    out = nc.dram_tensor("out", [_SL, 128, 2, _D], bf16, kind="ExternalOutput")

    with tile.TileContext(nc) as tc:
        with (
            tc.tile_pool(name="const", bufs=1) as cpool,
            tc.tile_pool(name="data", bufs=3) as dpool,
            tc.tile_pool(name="small", bufs=2) as spool,
            tc.tile_pool(name="psA", bufs=2, space="PSUM") as psa_pool,
            tc.tile_pool(name="psC", bufs=2, space="PSUM") as psc_pool,
            tc.tile_pool(name="psM", bufs=1, space="PSUM") as psm_pool,
            tc.tile_pool(name="psS", bufs=1, space="PSUM") as pss_pool,
            tc.tile_pool(name="psND", bufs=1, space="PSUM") as psnd_pool,
            tc.tile_pool(name="psW", bufs=1, space="PSUM") as psw_pool,
        ):
            wmup = cpool.tile([128, 128], bf16)
            nc.vector.memset(wmup[:], 0.5)
            ones32 = cpool.tile([32, 1], bf16)
            nc.vector.memset(ones32[:], 1.0)
            ident = cpool.tile([128, 128], f32)
            masks.make_identity(nc, ident[:])
            # band master: mband[k, c] = 1 iff c == k + 128.  Slicing cols
            # [144-16j : 272-16j] gives the [32k, 128r] band matrix with
            # 1 at r == k + 16j - 16 (rows outside [0,128) auto-dropped).
            mband = cpool.tile([32, 272], bf16)
            nc.gpsimd.memset(mband[:], 0.0)
            nc.gpsimd.affine_select(
                out=mband[:],
                in_=mband[:],
                compare_op=mybir.AluOpType.not_equal,
                fill=1.0,
                base=128,
                pattern=[[-1, 272]],
                channel_multiplier=1,
            )
            wt_sb = cpool.tile([128, 1024], bf16)
            nc.scalar.dma_start(wt_sb[:], wt[:, :])

            # PE p-state warm-up: harmless matmuls while input DMAs fill
            psW = psw_pool.tile([64, 128], f32)
            for _ in range(_NWARM):
                nc.tensor.matmul(
                    psW[:, :], wmup[:, 0:64], wmup[:, :],
                    start=True, stop=True, skip_group_check=True,
                )

            # queue input DMAs on the SP ring (pure DMA ring: backpressure
            # waits must not block compute dispatch).  With bufs=2 input
            # pools, slice s+2's DMA naturally waits for slice s's last
            # reader — keeping HBM arrivals slice-ordered and just-in-time
            # instead of all transfers round-robin-finishing together.
            xtm_sb = [
                dpool.tile([128, _N], bf16, tag="xtm", name=f"xtm_{s}")
                for s in range(_SL)
            ]
            xn_sb = [
                dpool.tile([128, _N], bf16, tag="xn", name=f"xn_{s}")
                for s in range(_SL)
            ]
            nc.sync.dma_start(xtm_sb[0][:, 0:2048], xtm[0, :, 0:2048])
            nc.sync.dma_start(xtm_sb[0][:, 2048:4096], xtm[0, :, 2048:4096])
            nc.sync.dma_start(xn_sb[0][:, :], xn[0, :, :])
            nc.sync.dma_start(xtm_sb[1][:, :], xtm[1, :, :])
            nc.sync.dma_start(xn_sb[1][:, :], xn[1, :, :])
            nc.sync.dma_start(xtm_sb[2][:, :], xtm[2, :, :])
            nc.sync.dma_start(xn_sb[2][:, :], xn[2, :, :])
            nc.sync.dma_start(xtm_sb[3][:, :], xtm[3, :, :])
            nc.sync.dma_start(xn_sb[3][:, :], xn[3, :, :])

            fin = {}

            def emit_finish_copy(s):
                """psC -> SBUF f32 copy (DVE), frees psC."""
                psC, outT_sb, rden = fin[s]
                nc.vector.tensor_copy(outT_sb[:, 0:255], psC[:, 0:255])

            def emit_finish_rest(s):
                """transpose to [n, d] -> normalize -> slice DMA out."""
                _, outT_sb, rden = fin.pop(s)
                psND = psnd_pool.tile([128, 256], f32, tag="psND")
                nc.tensor.transpose(
                    psND[0:128, 0:128], outT_sb[:, 0:128], ident[:, :]
                )
                nc.tensor.transpose(
                    psND[0:127, 128:256], outT_sb[:, 128:255], ident[:, :]
                )
                o_s = spool.tile([128, 2, _D], bf16, tag="o_s")
                nc.gpsimd.memset(o_s[:], 0.0)
                nc.vector.tensor_scalar(
                    o_s[:, 0, :], psND[0:128, 0:128], rden[0:128, 0:1],
                    None, mybir.AluOpType.mult,
                )
                nc.vector.tensor_scalar(
                    o_s[0:127, 1, :], psND[0:127, 128:256],
                    rden[0:127, 1:2], None, mybir.AluOpType.mult,
                )
                nc.scalar.dma_start(out[s, :, :, :], o_s[:, :, :])

            for s in range(_SL):
                if s > 0:
                    emit_finish_copy(s - 1)

                # ---- stage A: 2 concurrent accumulation chains, 64-wide ----
                # psA[64*ch + 32*kap + g, c] += wt[:,64m+32kap+g] . x[16c+m, :]
                psA = psa_pool.tile([128, 256], f32, tag="psA", name=f"psA_{s}")
                for m in range(16):
                    ch = m & 1
                    nc.tensor.matmul(
                        psA[64 * ch : 64 * ch + 64, :],
                        wt_sb[:, 64 * m : 64 * m + 64],
                        xtm_sb[s][:, 256 * m : 256 * m + 256],
                        start=(m < 2),
                        stop=(m >= 14),
                        tile_position=(0, 64 * ch),
                        skip_group_check=True,
                    )

                # previous slice's PE transposes fill the exp/fold latency gap
                if s > 0:
                    emit_finish_rest(s - 1)

                # ---- stage B: exp + fold + denominators ----
                # relocate the 4 (chain,kappa) psum blocks to partition base 0
                # during exp (DVE tensor_tensor needs equal SBUF bases)
                E = spool.tile([32, 1024], f32, tag="E")
                for q in range(4):
                    nc.scalar.activation(
                        E[:, 256 * q : 256 * q + 256],
                        psA[32 * q : 32 * q + 32, :],
                        mybir.ActivationFunctionType.Exp,
                    )

                psC = psc_pool.tile([128, 256], f32, tag="psC", name=f"psC_{s}")
                nc.vector.memset(psC[:], 0.0)

                # kappa products across chains: psA blocks are (ch,kap) =
                # q0:(0,0) q1:(0,1) q2:(1,0) q3:(1,1)
                P = spool.tile([32, 512], f32, tag="P")
                nc.vector.tensor_mul(P[:, 0:256], E[:, 0:256], E[:, 512:768])
                nc.vector.tensor_mul(P[:, 256:512], E[:, 256:512], E[:, 768:1024])
                e_kn = spool.tile([32, 256], bf16, tag="e_kn")
                nc.vector.tensor_mul(
                    e_kn[:, 0:255], P[:, 0:255], P[:, 257:512]
                )

                # denominators: ones-matmul row + PE transpose + reciprocal
                psM = psm_pool.tile([128, 258], f32, tag="psM")
                nc.tensor.matmul(
                    psM[0:1, 0:255], ones32[:, 0:1], e_kn[:, 0:255],
                    start=True, stop=True, skip_group_check=True,
                )
                den_sb = spool.tile([1, 256], f32, tag="den")
                nc.vector.tensor_copy(den_sb[0:1, 0:255], psM[0:1, 0:255])
                nc.tensor.transpose(
                    psM[0:128, 256:257], den_sb[0:1, 0:128], ident[0:1, 0:1]
                )
                nc.tensor.transpose(
                    psM[0:127, 257:258], den_sb[0:1, 128:255], ident[0:1, 0:1]
                )
                rden = spool.tile([128, 2], f32, tag="rden")
                nc.vector.reciprocal(rden[0:128, 0:1], psM[0:128, 256:257])
                nc.vector.reciprocal(rden[0:127, 1:2], psM[0:127, 257:258])

                # ---- S band matrix on PE: S[16j-16+k, 32j+c] = e[k, 8c+j-1]
                psS = pss_pool.tile([128, 288], f32, tag="psS")
                for j in range(9):
                    c0 = 1 if j == 0 else 0
                    c1 = 31 if j == 8 else 32
                    nc.tensor.matmul(
                        psS[:, 32 * j + c0 : 32 * j + c1],
                        mband[:, 144 - 16 * j : 272 - 16 * j],
                        e_kn[:, 8 * c0 + j - 1 : 8 * (c1 - 1) + j : 8],
                        start=True,
                        stop=True,
                        skip_group_check=True,
                    )
                S_sb = spool.tile([128, 288], bf16, tag="S")
                nc.vector.tensor_copy(S_sb[:, 1:287], psS[:, 1:287])

                # ---- stage C: psC[d, n] += xn_chunk^T @ S_cols ----
                for c in range(_NCH):
                    j0 = 1 if c == 0 else 0
                    j1 = 8 if c == _NCH - 1 else 9
                    nc.tensor.matmul(
                        psC[:, 8 * c - 1 + j0 : 8 * c - 1 + j1],
                        xn_sb[s][:, 128 * c : 128 * c + 128],
                        S_sb[:, 32 * j0 + c : 32 * (j1 - 1) + c + 1 : 32],
                        start=False,
                        stop=(c == _NCH - 1),
                        skip_group_check=True,
                    )

                outT_sb = spool.tile([128, 256], f32, tag="outT")
                fin[s] = (psC, outT_sb, rden)

            emit_finish_copy(_SL - 1)
            emit_finish_rest(_SL - 1)

    nc.compile()
    return nc


def _get_program():
    if "nc" not in _prog_cache:
        _prog_cache["nc"] = _build_program()
    return _prog_cache["nc"]


def _host_inputs(x, W_gate):
    bf16 = ml_dtypes.bfloat16
    x = np.asarray(x, dtype=np.float32)
    W = np.asarray(W_gate, dtype=np.float32)
    # wt[d, 64m + 32kap + g] = W_gate[g, (m+16kap)*128 + d]
    wt_host = np.ascontiguousarray(
        W.reshape(_K, 2, 16, _D).transpose(3, 2, 1, 0).reshape(_D, 1024)
    ).astype(bf16)
    xb = x.astype(bf16)
    in_maps = []
    for core in range(_NC):
        xn = np.empty((_SL, 128, _N), dtype=bf16)
        xtm = np.empty((_SL, 128, _N), dtype=bf16)
        for si in range(_SL):
            p = core * _SL + si
            b, h = p // _H, p % _H
            xs = xb[b, :, h, :]  # [4096, 128]
            # xn[si, p_, 128c + d] = x[128c + p_, d]
            xn[si] = (
                xs.reshape(_NCH, 128, _D).transpose(1, 0, 2).reshape(128, -1)
            )
            # xtm[si, d, 256m + c] = x[16c + m, d]
            xtm[si] = (
                xs.reshape(256, 16, _D).transpose(2, 1, 0).reshape(128, -1)
            )
        in_maps.append({"xn": xn, "xtm": xtm, "wt": wt_host})
    return in_maps


def _assemble(results):
    out = np.empty((_B, _NB, _H, _D), dtype=np.float32)
    for core in range(_NC):
        o = np.asarray(results[core]["out"]).astype(np.float32)  # [4,128,2,128]
        for si in range(_SL):
            p = core * _SL + si
            b, h = p // _H, p % _H
            out[b, 0:128, h, :] = o[si, :, 0, :]
            out[b, 128:_NB, h, :] = o[si, 0:127, 1, :]
    return out


def _install_trace_hooks():
    """Shim the axon NTFF profile hook (missing in this image) so
    run_bass_kernel_spmd(trace=True) can collect a HW profile, and neuter
    the artifact upload (zero-egress container)."""
    import contextlib
    import ctypes
    import types

    try:
        from antenv.axon_hooks import get_axon_ntff_profile_hook  # noqa: F401

        return
    except ImportError:
        pass

    lib = ctypes.CDLL("/opt/axon/libaxon_pjrt.so")
    if not hasattr(lib, "axon_start_nrt_profile"):
        return
    lib.axon_start_nrt_profile.argtypes = [
        ctypes.POINTER(ctypes.c_int64),
        ctypes.c_size_t,
    ]
    lib.axon_start_nrt_profile.restype = ctypes.c_int64
    lib.axon_stop_nrt_profile.argtypes = [ctypes.c_char_p]
    lib.axon_stop_nrt_profile.restype = ctypes.c_int64

    @contextlib.contextmanager
    def _hook(output_dir, device_ids):
        import jax

        jax.devices()
        if device_ids:
            ids = (ctypes.c_int64 * len(device_ids))(*device_ids)
            rc = lib.axon_start_nrt_profile(ids, len(device_ids))
        else:
            rc = lib.axon_start_nrt_profile(None, 0)
        if rc != 0:
            raise RuntimeError(f"axon_start_nrt_profile rc={rc}")
        try:
            yield
        finally:
            n = lib.axon_stop_nrt_profile(str(output_dir).encode())
            print(f"profile: {n} file(s) written to {output_dir}")

    mod = types.ModuleType("antenv.axon_hooks")
    mod.get_axon_ntff_profile_hook = lambda: _hook
    mod.set_axon_ntff_profile_hook = lambda h: None
    sys.modules["antenv.axon_hooks"] = mod

    from concourse import bass_utils as bu

    bu.upload_artifacts = lambda tmpdir: tmpdir


def run(x, W_gate, trace=False, **kw):
    from concourse.bass_utils import run_bass_kernel_spmd

    if trace:
        _install_trace_hooks()
    nc = _get_program()
    in_maps = _host_inputs(x, W_gate)
    res = run_bass_kernel_spmd(nc, in_maps, list(range(_NC)), trace=trace, **kw)
    return _assemble(res.results), res


def kernel(x, W_gate):
    out, _ = run(x, W_gate)
    return out


# revision 32
# speedup vs baseline: 1.2101x; 1.0737x over previous
"""CompressKV gating kernel for 8 Trainium2 NeuronCores (v3).

Reference computation (per batch b, head h):
    x_s = x[b, :, h, :]                                  # [N=4096, D=128]
    windows n = 0..254, rows r = 16n + k, k = 0..31
    logits[n, g] = sum_{k,d} x_s[16n+k, d] * W[g, k, d]  # W = W_gate.reshape(32,32,128)
    gate = softmax_g(logits)
    out[n, d] = sum_k gate[n, k] * x_s[16n+k, d]

Sharding: B*H = 32 (b,h) slices, 4 per core, no cross-core communication.
Host pre-packs x per core in two bf16 layouts:
  xn : row-chunked native   [4, 128(p), 32(c)*128(d)]  (chunk c = rows 128c+p)
  xtm: d-major m-interleaved [4, 128(d), 16(m)*256(c)] (col 256m+c = row 16c+m)
plus wt[d, 64m+32kap+g] = W_gate[g, (m+16kap)*128+d].

v3 vs the 56us baseline:
  - stage A packs (kappa,gate)=64 outputs per streamed column via two
    interleaved PE column-tile accumulation chains; the m-interleaved
    xtm layout makes every rhs stream contiguous: 16 matmuls x 256 cols
    per slice (half the baseline's streamed columns).
    logits[n,g] = acc[(0,g),n] + acc[(1,g),n+1] folded as exp-products.
  - exp relocates the 4 (chain,kappa) psum blocks to partition base 0
    (DVE tensor_tensor requires equal SBUF base partitions).
  - bf16 outputs leave in one contiguous DMA per slice as it finishes.
  - input dispatch split across both HWDGE rings (SP + ACT).
  - PE warm-up matmuls during the initial DMA fill raise the p-state.
"""

import sys

import numpy as np

for _p in ("/opt/trn_rl_repo", "/opt/pypackages"):
    if _p not in sys.path:
        sys.path.append(_p)

import ml_dtypes

_B, _N, _H, _D = 2, 4096, 16, 128
_K = 32          # window (kernel) size
_ST = 16         # stride
_NB = 255        # num windows
_NC = 8          # cores
_SL = 4          # (b,h) slices per core
_NCH = 32        # 128-row chunks per slice
_NWARM = 32      # PE p-state warm-up matmuls

_prog_cache = {}


def _build_program():
    import concourse.mybir as mybir
    from concourse import bacc, masks, tile

    f32 = mybir.dt.float32
    bf16 = mybir.dt.bfloat16

    nc = bacc.Bacc()
    xn = nc.dram_tensor("xn", [_SL, 128, _N], bf16, kind="ExternalInput")
    xtm = nc.dram_tensor("xtm", [_SL, 128, _N], bf16, kind="ExternalInput")
    wt = nc.dram_tensor("wt", [128, 1024], bf16, kind="ExternalInput")
    # out[s, p, half, d] = out_slice_s[window 128*half + p, d]
    out = nc.dram_tensor("out", [_SL, 128, 2, _D], bf16, kind="ExternalOutput")

    with tile.TileContext(nc) as tc:
        with (
            tc.tile_pool(name="const", bufs=1) as cpool,
            tc.tile_pool(name="dataT", bufs=2) as dtpool,
            tc.tile_pool(name="dataN", bufs=3) as dnpool,
            tc.tile_pool(name="small", bufs=2) as spool,
            tc.tile_pool(name="psA", bufs=2, space="PSUM") as psa_pool,
            tc.tile_pool(name="psC", bufs=2, space="PSUM") as psc_pool,
            tc.tile_pool(name="psM", bufs=1, space="PSUM") as psm_pool,
            tc.tile_pool(name="psS", bufs=1, space="PSUM") as pss_pool,
            tc.tile_pool(name="psND", bufs=1, space="PSUM") as psnd_pool,
            tc.tile_pool(name="psW", bufs=1, space="PSUM") as psw_pool,
        ):
            wmup = cpool.tile([128, 128], bf16)
            nc.vector.memset(wmup[:], 0.5)
            ones32 = cpool.tile([32, 1], bf16)
            nc.vector.memset(ones32[:], 1.0)
            ident = cpool.tile([128, 128], f32)
            masks.make_identity(nc, ident[:])
            # band master: mband[k, c] = 1 iff c == k + 128.  Slicing cols
            # [144-16j : 272-16j] gives the [32k, 128r] band matrix with
            # 1 at r == k + 16j - 16 (rows outside [0,128) auto-dropped).
            mband = cpool.tile([32, 272], bf16)
            nc.gpsimd.memset(mband[:], 0.0)
            nc.gpsimd.affine_select(
                out=mband[:],
                in_=mband[:],
                compare_op=mybir.AluOpType.not_equal,
                fill=1.0,
                base=128,
                pattern=[[-1, 272]],
                channel_multiplier=1,
            )
            wt_sb = cpool.tile([128, 1024], bf16)
            nc.scalar.dma_start(wt_sb[:], wt[:, :])

            # PE p-state warm-up: harmless matmuls while input DMAs fill
            psW = psw_pool.tile([64, 128], f32)
            for _ in range(_NWARM):
                nc.tensor.matmul(
                    psW[:, :], wmup[:, 0:64], wmup[:, :],
                    start=True, stop=True, skip_group_check=True,
                )

            # queue input DMAs on the SP ring (pure DMA ring: backpressure
            # waits must not block compute dispatch).  With bufs=2 input
            # pools, slice s+2's DMA naturally waits for slice s's last
            # reader — keeping HBM arrivals slice-ordered and just-in-time
            # instead of all transfers round-robin-finishing together.
            xtm_sb = [
                dtpool.tile([128, _N], bf16, tag="xtm", name=f"xtm_{s}")
                for s in range(_SL)
            ]
            xn_sb = [
                dnpool.tile([128, _N], bf16, tag="xn", name=f"xn_{s}")
                for s in range(_SL)
            ]
            nc.sync.dma_start(xtm_sb[0][:, 0:2048], xtm[0, :, 0:2048])
            nc.sync.dma_start(xtm_sb[0][:, 2048:4096], xtm[0, :, 2048:4096])
            nc.sync.dma_start(xtm_sb[1][:, :], xtm[1, :, :])
            nc.sync.dma_start(xn_sb[0][:, :], xn[0, :, :])
            nc.sync.dma_start(xn_sb[1][:, :], xn[1, :, :])
            nc.sync.dma_start(xtm_sb[2][:, :], xtm[2, :, :])
            nc.sync.dma_start(xn_sb[2][:, :], xn[2, :, :])
            nc.sync.dma_start(xtm_sb[3][:, :], xtm[3, :, :])
            nc.sync.dma_start(xn_sb[3][:, :], xn[3, :, :])

            fin = {}

            def emit_finish_copy(s):
                """psC -> SBUF f32 copy (DVE), frees psC."""
                psC, outT_sb, rden = fin[s]
                nc.vector.tensor_copy(outT_sb[:, 0:255], psC[:, 0:255])

            def emit_finish_rest(s):
                """transpose to [n, d] -> normalize -> slice DMA out."""
                _, outT_sb, rden = fin.pop(s)
                psND = psnd_pool.tile([128, 256], f32, tag="psND")
                nc.tensor.transpose(
                    psND[0:128, 0:128], outT_sb[:, 0:128], ident[:, :]
                )
                nc.tensor.transpose(
                    psND[0:127, 128:256], outT_sb[:, 128:255], ident[:, :]
                )
                o_s = spool.tile([128, 2, _D], bf16, tag="o_s")
                nc.gpsimd.memset(o_s[:], 0.0)
                nc.vector.tensor_scalar(
                    o_s[:, 0, :], psND[0:128, 0:128], rden[0:128, 0:1],
                    None, mybir.AluOpType.mult,
                )
                nc.vector.tensor_scalar(
                    o_s[0:127, 1, :], psND[0:127, 128:256],
                    rden[0:127, 1:2], None, mybir.AluOpType.mult,
                )
                nc.scalar.dma_start(out[s, :, :, :], o_s[:, :, :])

            astate = {}

            def emit_A(s):
                # ---- stage A: 2 concurrent accumulation chains, 64-wide ---
                # psA[64*ch + 32*kap + g, c] += wt[:,64m+32kap+g] . x[16c+m,:]
                psA = psa_pool.tile([128, 256], f32, tag="psA", name=f"psA_{s}")
                for m in range(16):
                    ch = m & 1
                    nc.tensor.matmul(
                        psA[64 * ch : 64 * ch + 64, :],
                        wt_sb[:, 64 * m : 64 * m + 64],
                        xtm_sb[s][:, 256 * m : 256 * m + 256],
                        start=(m < 2),
                        stop=(m >= 14),
                        tile_position=(0, 64 * ch),
                        skip_group_check=True,
                    )
                astate[s] = psA

            # software pipeline: A(s+1) fills the PE while slice s's
            # exp/fold runs on ACT/DVE
            emit_A(0)
            for s in range(_SL):
                if s + 1 < _SL:
                    emit_A(s + 1)
                psA = astate.pop(s)

                # ---- stage B: exp + fold ----
                # relocate the 4 (chain,kappa) psum blocks to partition
                # base 0 during exp (DVE needs equal SBUF bases)
                E = spool.tile([32, 1024], f32, tag="E")
                for q in range(4):
                    nc.scalar.activation(
                        E[:, 256 * q : 256 * q + 256],
                        psA[32 * q : 32 * q + 32, :],
                        mybir.ActivationFunctionType.Exp,
                    )

                # previous slice's PE transposes + out DMA (emitted after
                # the exps so the ACT ring prioritizes the critical exp)
                if s > 0:
                    emit_finish_rest(s - 1)

                psC = psc_pool.tile([128, 256], f32, tag="psC", name=f"psC_{s}")
                nc.vector.memset(psC[:], 0.0)

                # kappa products across chains: psA blocks are (ch,kap) =
                # q0:(0,0) q1:(0,1) q2:(1,0) q3:(1,1)
                P = spool.tile([32, 512], f32, tag="P")
                nc.vector.tensor_mul(P[:, 0:256], E[:, 0:256], E[:, 512:768])
                nc.vector.tensor_mul(P[:, 256:512], E[:, 256:512], E[:, 768:1024])
                e_kn = spool.tile([32, 256], bf16, tag="e_kn")
                nc.vector.tensor_mul(
                    e_kn[:, 0:255], P[:, 0:255], P[:, 257:512]
                )

                # ---- S band matrix on PE: S[16j-16+k, 32j+c] = e[k, 8c+j-1]
                psS = pss_pool.tile([128, 288], f32, tag="psS")
                for j in range(9):
                    c0 = 1 if j == 0 else 0
                    c1 = 31 if j == 8 else 32
                    nc.tensor.matmul(
                        psS[:, 32 * j + c0 : 32 * j + c1],
                        mband[:, 144 - 16 * j : 272 - 16 * j],
                        e_kn[:, 8 * c0 + j - 1 : 8 * (c1 - 1) + j : 8],
                        start=True,
                        stop=True,
                        skip_group_check=True,
                    )
                S_sb = spool.tile([128, 288], bf16, tag="S")
                nc.vector.tensor_copy(S_sb[:, 1:287], psS[:, 1:287])

                # ---- stage C: psC[d, n] += xn_chunk^T @ S_cols ----
                for c in range(_NCH):
                    j0 = 1 if c == 0 else 0
                    j1 = 8 if c == _NCH - 1 else 9
                    nc.tensor.matmul(
                        psC[:, 8 * c - 1 + j0 : 8 * c - 1 + j1],
                        xn_sb[s][:, 128 * c : 128 * c + 128],
                        S_sb[:, 32 * j0 + c : 32 * (j1 - 1) + c + 1 : 32],
                        start=False,
                        stop=(c == _NCH - 1),
                        skip_group_check=True,
                    )

                # denominators after C (rden is only needed at finish time)
                psM = psm_pool.tile([128, 258], f32, tag="psM")
                nc.tensor.matmul(
                    psM[0:1, 0:255], ones32[:, 0:1], e_kn[:, 0:255],
                    start=True, stop=True, skip_group_check=True,
                )
                den_sb = spool.tile([1, 256], f32, tag="den")
                nc.vector.tensor_copy(den_sb[0:1, 0:255], psM[0:1, 0:255])
                nc.tensor.transpose(
                    psM[0:128, 256:257], den_sb[0:1, 0:128], ident[0:1, 0:1]
                )
                nc.tensor.transpose(
                    psM[0:127, 257:258], den_sb[0:1, 128:255], ident[0:1, 0:1]
                )
                rden = spool.tile([128, 2], f32, tag="rden")
                nc.vector.reciprocal(rden[0:128, 0:1], psM[0:128, 256:257])
                nc.vector.reciprocal(rden[0:127, 1:2], psM[0:127, 257:258])

                outT_sb = spool.tile([128, 256], f32, tag="outT")
                fin[s] = (psC, outT_sb, rden)
                emit_finish_copy(s)

            emit_finish_rest(_SL - 1)

    nc.compile()
    return nc


def _get_program():
    if "nc" not in _prog_cache:
        _prog_cache["nc"] = _build_program()
    return _prog_cache["nc"]


def _host_inputs(x, W_gate):
    bf16 = ml_dtypes.bfloat16
    x = np.asarray(x, dtype=np.float32)
    W = np.asarray(W_gate, dtype=np.float32)
    # wt[d, 64m + 32kap + g] = W_gate[g, (m+16kap)*128 + d]
    wt_host = np.ascontiguousarray(
        W.reshape(_K, 2, 16, _D).transpose(3, 2, 1, 0).reshape(_D, 1024)
    ).astype(bf16)
    xb = x.astype(bf16)
    in_maps = []
    for core in range(_NC):
        xn = np.empty((_SL, 128, _N), dtype=bf16)
        xtm = np.empty((_SL, 128, _N), dtype=bf16)
        for si in range(_SL):
            p = core * _SL + si
            b, h = p // _H, p % _H
            xs = xb[b, :, h, :]  # [4096, 128]
            # xn[si, p_, 128c + d] = x[128c + p_, d]
            xn[si] = (
                xs.reshape(_NCH, 128, _D).transpose(1, 0, 2).reshape(128, -1)
            )
            # xtm[si, d, 256m + c] = x[16c + m, d]
            xtm[si] = (
                xs.reshape(256, 16, _D).transpose(2, 1, 0).reshape(128, -1)
            )
        in_maps.append({"xn": xn, "xtm": xtm, "wt": wt_host})
    return in_maps


def _assemble(results):
    out = np.empty((_B, _NB, _H, _D), dtype=np.float32)
    for core in range(_NC):
        o = np.asarray(results[core]["out"]).astype(np.float32)  # [4,128,2,128]
        for si in range(_SL):
            p = core * _SL + si
            b, h = p // _H, p % _H
            out[b, 0:128, h, :] = o[si, :, 0, :]
            out[b, 128:_NB, h, :] = o[si, 0:127, 1, :]
    return out


def _install_trace_hooks():
    """Shim the axon NTFF profile hook (missing in this image) so
    run_bass_kernel_spmd(trace=True) can collect a HW profile, and neuter
    the artifact upload (zero-egress container)."""
    import contextlib
    import ctypes
    import types

    try:
        from antenv.axon_hooks import get_axon_ntff_profile_hook  # noqa: F401

        return
    except ImportError:
        pass

    lib = ctypes.CDLL("/opt/axon/libaxon_pjrt.so")
    if not hasattr(lib, "axon_start_nrt_profile"):
        return
    lib.axon_start_nrt_profile.argtypes = [
        ctypes.POINTER(ctypes.c_int64),
        ctypes.c_size_t,
    ]
    lib.axon_start_nrt_profile.restype = ctypes.c_int64
    lib.axon_stop_nrt_profile.argtypes = [ctypes.c_char_p]
    lib.axon_stop_nrt_profile.restype = ctypes.c_int64

    @contextlib.contextmanager
    def _hook(output_dir, device_ids):
        import jax

        jax.devices()
        if device_ids:
            ids = (ctypes.c_int64 * len(device_ids))(*device_ids)
            rc = lib.axon_start_nrt_profile(ids, len(device_ids))
        else:
            rc = lib.axon_start_nrt_profile(None, 0)
        if rc != 0:
            raise RuntimeError(f"axon_start_nrt_profile rc={rc}")
        try:
            yield
        finally:
            n = lib.axon_stop_nrt_profile(str(output_dir).encode())
            print(f"profile: {n} file(s) written to {output_dir}")

    mod = types.ModuleType("antenv.axon_hooks")
    mod.get_axon_ntff_profile_hook = lambda: _hook
    mod.set_axon_ntff_profile_hook = lambda h: None
    sys.modules["antenv.axon_hooks"] = mod

    from concourse import bass_utils as bu

    bu.upload_artifacts = lambda tmpdir: tmpdir


def run(x, W_gate, trace=False, **kw):
    from concourse.bass_utils import run_bass_kernel_spmd

    if trace:
        _install_trace_hooks()
    nc = _get_program()
    in_maps = _host_inputs(x, W_gate)
    res = run_bass_kernel_spmd(nc, in_maps, list(range(_NC)), trace=trace, **kw)
    return _assemble(res.results), res


def kernel(x, W_gate):
    out, _ = run(x, W_gate)
    return out
